# revision 13
# baseline (speedup 1.0000x reference)
"""Trainium2 Bass kernel for nn_EvalEig (radial Numerov eigen-eval).

Self-contained: hardcodes shapes from the problem spec.
  input : init_energy (4096,) float32
  output: tuple(energy+delta (4096,3), u_zero (100,4096,3), u_infty (900,4096,3))

Sharding: data-parallel over the energy axis, 512 energies per core x 8 cores.
Per core: 4 groups x 128 partitions; l=0..2 and the radial axis in SBUF free
dims.  The 894-step (and 94-step) Numerov recurrences are parallelized with a
chunked basis-solution method: 71 chunks x 14 steps, 4 basis solutions per
chunk evolved together, then a sequential 4x4 fold across chunk boundaries and
a batched linear recombination.
"""
import os
import sys
import functools
import numpy as np

if "/opt/trn_rl_repo" not in sys.path:
    sys.path.insert(0, "/opt/trn_rl_repo")

import concourse.bass as bass
import concourse.bacc as bacc
import concourse.mybir as mybir
from concourse import tile
from concourse import bass_utils

F32 = mybir.dt.float32
AF = mybir.ActivationFunctionType
OP = mybir.AluOpType

# problem constants
RM, RN, RD, L = 100.0, 1000, 0.1, 3
NE_FULL, NCORES = 4096, 8
NE_CORE = NE_FULL // NCORES          # 512
P, G = 128, NE_CORE // 128           # 128 partitions x 4 groups
T, C_OUT, C_IN = 14, 64, 7           # chunking of the two scans
C = C_OUT + C_IN                     # 71
NSLOT = 1008                         # coefficient slots (out 0..895, in 896..993)
NV = 904                             # v buffer (positions 0..900 used)
NZ = 104
C2H = 8                              # C2 out-chunk batch size

C_ = float(np.float32(3.0 / 40.0 * RD * RD))
K1 = float(np.float32(13.0 / 15.0 * RD * RD))
K2 = float(np.float32(7.0 / 60.0 * RD * RD))
W7 = float(np.float32(7 * 2 * RD / 45))
W32 = float(np.float32(32 * 2 * RD / 45))
W12 = float(np.float32(12 * 2 * RD / 45))
WD = [float(np.float32(w / (12 * RD))) for w in (25.0, -48.0, 36.0, -16.0, 3.0)]


def build_group(nc, pools, g, dram, consts):
    """Emit instructions for one group of 128 energies."""
    (pA, pB, pC, pD) = pools
    blob_t = consts
    arep_t = blob_t[:, 0:3000].rearrange("p (l r) -> p l r", r=RN)
    e_col = blob_t[:, 3000 + g:3001 + g]
    uii_g = blob_t[:, 3004 + g * 15: 3004 + (g + 1) * 15].rearrange(
        "p (l c) -> p l c", c=5)
    uzi_t = blob_t[:, 3064:3079].rearrange("p (l c) -> p l c", c=5)

    # ---------------- phase A: factor, reciprocal-denominator, coefficients
    fac = pA.tile([P, L, RN], F32, tag="fac")
    nc.vector.tensor_scalar(fac[:], arep_t, e_col, None, op0=OP.subtract)
    dnm = pA.tile([P, L, RN], F32, tag="dnm")
    nc.vector.tensor_scalar(dnm[:], fac[:], -C_, 1.0, op0=OP.mult, op1=OP.add)
    rdf = pA.tile([P, L, RN], F32, tag="rdf")
    # fast seed into rdf, then in-place Newton step (identical APs: the
    # elementwise stream reads each element before overwriting it)
    from concourse.dve_ops import RECIPROCAL_APPROX_NR
    rdf_f = rdf[:].rearrange("p l r -> p (l r)")
    dnm_f = dnm[:].rearrange("p l r -> p (l r)")
    nc.vector.reciprocal_approx_fast(out=rdf_f, in_=dnm_f)
    nc.vector._custom_dve(RECIPROCAL_APPROX_NR, out=rdf_f, in0=dnm_f,
                          in1=rdf_f, s0=2.0)

    coefs = []
    for name, foff, kap, alp in (
            ("ca", 0, C_, -1.0), ("cb", 1, K1, 2.0),
            ("cd2", 2, K2, -2.0), ("cd3", 3, K1, 2.0)):
        cf = pA.tile([P, L, NSLOT], F32, tag=name)
        # out-scan slots s=0..895 (step i=s+1, reversed factor):
        #   f_k = fac[998-foff-s], rd = rdf[994-s]
        hi = 998 - foff
        nc.vector.tensor_tensor(
            cf[:, :, 0:896],
            fac[:, :, hi:hi - 896:-1],
            rdf[:, :, 994:98:-1],
            op=OP.mult)
        # in-scan slots 896+s', s'=0..97: f_k = fac[1+foff+s'], rd = rdf[5+s']
        lo = 1 + foff
        nc.vector.tensor_tensor(
            cf[:, :, 896:994],
            fac[:, :, lo:lo + 98],
            rdf[:, :, 5:103],
            op=OP.mult)
        # alpha/kappa fold: cf = cf*kappa + alpha (in place, ACT)
        nc.scalar.activation(cf[:, :, 0:994], cf[:, :, 0:994], AF.Copy,
                             bias=alp, scale=kap)
        coefs.append(cf)

    # ---------------- phase B: 4 basis solutions, all chunks in parallel
    BB = pB.tile([P, 4, L, C, T + 4], F32, tag="BB")
    nc.gpsimd.memset(BB[:, :, :, :, 0:4], 0.0)
    for k in range(4):
        nc.gpsimd.memset(BB[:, k, :, :, k:k + 1], 1.0)
    for t in range(T):
        Ps = []
        for k in range(4):
            pk = pB.tile([P, 4, L, C], F32, tag=f"P{k}")
            win = BB[:, :, :, :, t + k]
            cap = coefs[k][:, :, t:t + C * T:T]          # [P, L, C] stride T
            cap = cap.unsqueeze(1).broadcast_to((P, 4, L, C))
            nc.vector.tensor_tensor(pk[:], win, cap, op=OP.mult)
            Ps.append(pk)
        nc.gpsimd.tensor_tensor(Ps[0][:], Ps[0][:], Ps[1][:], op=OP.add)
        nc.gpsimd.tensor_tensor(Ps[2][:], Ps[2][:], Ps[3][:], op=OP.add)
        nc.vector.tensor_tensor(BB[:, :, :, :, t + 4], Ps[0][:], Ps[2][:],
                                op=OP.add)

    # ---------------- phase C1: sequential fold of chunk-boundary windows
    Wt = pC.tile([P, L, C + 1, 4], F32, tag="W")
    nc.gpsimd.tensor_copy(Wt[:, :, 0, :], uii_g[:, :, 1:5])
    nc.gpsimd.tensor_copy(Wt[:, :, C_OUT, :], uzi_t[:, :, 1:5])
    for j in list(range(C_OUT - 1)) + list(range(C_OUT, C - 1)):
        pm = pC.tile([P, L, 4, 4], F32, tag=f"PM{j % 2}")
        # PM[l, m, k] = BB[k, l, j, T+m] * W[j][k]
        in0 = BB[:, :, :, j, T:T + 4]                     # [P, 4k, L, 4m]
        w_b = Wt[:, :, j, :].rearrange("p l k -> p k l") \
            .unsqueeze(3).broadcast_to((P, 4, L, 4))
        pm_t = pm[:].rearrange("p l m k -> p k l m")
        nc.vector.tensor_tensor(pm_t, in0, w_b, op=OP.mult)
        nc.vector.tensor_reduce(Wt[:, :, j + 1, :], pm[:],
                                axis=mybir.AxisListType.X, op=OP.add)

    # ---------------- phase C2: recombination into v / z
    v_full = pC.tile([P, L, NV], F32, tag="v")
    v = v_full[:]
    z = pC.tile([P, L, NZ], F32, tag="z")
    nc.gpsimd.tensor_copy(v[:, :, 0:5], uii_g[:, :, 0:5])
    nc.gpsimd.tensor_copy(z[:, :, 0:5], uzi_t[:, :, 0:5])
    for h in range(C_OUT // C2H):
        j0 = h * C2H
        Qs = []
        for k in range(4):
            qk = pC.tile([P, L, C2H, T], F32, tag=f"Q{k}")
            in0 = BB[:, k, :, j0:j0 + C2H, 4:4 + T]       # [P, L, C2H, T]
            w_b = Wt[:, :, j0:j0 + C2H, k].unsqueeze(3) \
                .broadcast_to((P, L, C2H, T))
            nc.vector.tensor_tensor(qk[:], in0, w_b, op=OP.mult)
            Qs.append(qk)
        nc.gpsimd.tensor_tensor(Qs[0][:], Qs[0][:], Qs[1][:], op=OP.add)
        nc.gpsimd.tensor_tensor(Qs[2][:], Qs[2][:], Qs[3][:], op=OP.add)
        out_ap = v[:, :, 5 + j0 * T: 5 + (j0 + C2H) * T] \
            .rearrange("p l (j t) -> p l j t", t=T)
        nc.vector.tensor_tensor(out_ap, Qs[0][:], Qs[2][:], op=OP.add)
    # in-scan chunks
    Qs = []
    for k in range(4):
        qk = pC.tile([P, L, C2H, T], F32, tag=f"Q{k}")
        qk = qk[:, :, :C_IN, :]
        in0 = BB[:, k, :, C_OUT:C, 4:4 + T]
        w_b = Wt[:, :, C_OUT:C, k].unsqueeze(3).broadcast_to((P, L, C_IN, T))
        nc.vector.tensor_tensor(qk, in0, w_b, op=OP.mult)
        Qs.append(qk)
    nc.gpsimd.tensor_tensor(Qs[0], Qs[0], Qs[1], op=OP.add)
    nc.gpsimd.tensor_tensor(Qs[2], Qs[2], Qs[3], op=OP.add)
    out_ap = z[:, :, 5:5 + C_IN * T].rearrange("p l (j t) -> p l j t", t=T)
    nc.vector.tensor_tensor(out_ap, Qs[0], Qs[2], op=OP.add)

    # ---------------- phase D: outputs O/Oz, integrals, derivatives
    O = pD.tile([P, L, 900], F32, tag="O")
    Oz = pD.tile([P, L, 100], F32, tag="Oz")
    nc.gpsimd.tensor_copy(O[:, :, 0:5], v[:, :, 894:899])
    nc.gpsimd.tensor_copy(O[:, :, 5:900], v[:, :, 898:3:-1])
    nc.gpsimd.tensor_copy(Oz[:, :, 0:5], z[:, :, 0:5])
    nc.gpsimd.tensor_copy(Oz[:, :, 5:100], z[:, :, 4:99])

    ints = pD.tile([P, 32], F32, tag="ints")
    scrI = pD.tile([P, 224], F32, tag="scrI")
    scrQ = pD.tile([P, 224], F32, tag="scrQ")

    def quart_term(dst_col, src_ap, w):
        # sum of w * (src^2)^2 over the strided set (all on DVE: the
        # TensorScalar ISA struct only fits one sync-wait slot)
        n = src_ap.shape[-1]
        sq = scrQ[:, 0:n]
        nc.vector.tensor_tensor(sq, src_ap, src_ap, op=OP.mult)
        nc.vector.scalar_tensor_tensor(
            scrI[:, 0:n], sq, w, sq,
            op0=OP.mult, op1=OP.mult, accum_out=ints[:, dst_col:dst_col + 1])

    def lin_term(dst_col, src_ap, w):
        # sum of w * src^2:  (src*w)*src, reduce-add
        nc.vector.scalar_tensor_tensor(
            scrI[:, 0:src_ap.shape[-1]], src_ap, w, src_ap,
            op0=OP.mult, op1=OP.mult, accum_out=ints[:, dst_col:dst_col + 1])

    for li in range(L):
        base = li * 5
        quart_term(base + 0, O[:, li, 0:893:4], W7)
        quart_term(base + 1, O[:, li, 1:894:4], W32)
        lin_term(base + 2, O[:, li, 2:895:4], W12)
        quart_term(base + 3, O[:, li, 3:896:4], W32)
        quart_term(base + 4, O[:, li, 4:897:4], W7)
        basez = 15 + li * 5
        quart_term(basez + 0, Oz[:, li, 0:93:4], W7)
        quart_term(basez + 1, Oz[:, li, 1:94:4], W32)
        lin_term(basez + 2, Oz[:, li, 2:95:4], W12)
        quart_term(basez + 3, Oz[:, li, 3:96:4], W32)
        quart_term(basez + 4, Oz[:, li, 4:97:4], W7)

    iout = pD.tile([P, L], F32, tag="iout")
    iin = pD.tile([P, L], F32, tag="iin")
    nc.vector.tensor_reduce(iout[:], ints[:, 0:15].rearrange(
        "p (l w) -> p l w", w=5), axis=mybir.AxisListType.X, op=OP.add)
    nc.vector.tensor_reduce(iin[:], ints[:, 15:30].rearrange(
        "p (l w) -> p l w", w=5), axis=mybir.AxisListType.X, op=OP.add)

    # derivative numerators (with 1/(12 RD) folded into weights)
    dnum = pD.tile([P, L], F32, tag=f"dnum{g}")
    dzn = pD.tile([P, L], F32, tag="dzn")
    nc.vector.tensor_scalar(dnum[:], O[:, :, 0], WD[0], None, op0=OP.mult)
    for q in range(1, 5):
        nc.vector.scalar_tensor_tensor(dnum[:], O[:, :, q], WD[q], dnum[:],
                                       op0=OP.mult, op1=OP.add)
    nc.vector.tensor_scalar(dzn[:], Oz[:, :, 99], WD[0], None, op0=OP.mult)
    for q in range(1, 5):
        nc.vector.scalar_tensor_tensor(dzn[:], Oz[:, :, 99 - q], WD[q], dzn[:],
                                       op0=OP.mult, op1=OP.add)

    # lfunc_in = dzn / Oz[99];  dene = iin/Oz[99]^2 + iout/O[0]^2
    rz = pD.tile([P, L], F32, tag="rz")
    ri0 = pD.tile([P, L], F32, tag="ri0")
    scrD = pD.tile([P, L], F32, tag="scrD")
    nc.vector.reciprocal_approx_accurate(out=rz[:], in_=Oz[:, :, 99],
                                         scratch=scrD[:])
    nc.vector.reciprocal_approx_accurate(out=ri0[:], in_=O[:, :, 0],
                                         scratch=scrD[:])
    lfin = pD.tile([P, L], F32, tag="lfin")
    nc.vector.tensor_tensor(lfin[:], dzn[:], rz[:], op=OP.mult)
    t1 = pD.tile([P, L], F32, tag="t1")
    t2 = pD.tile([P, L], F32, tag="t2")
    nc.vector.tensor_tensor(t1[:], rz[:], rz[:], op=OP.mult)
    nc.vector.tensor_tensor(t1[:], t1[:], iin[:], op=OP.mult)
    nc.vector.tensor_tensor(t2[:], ri0[:], ri0[:], op=OP.mult)
    nc.vector.tensor_tensor(t2[:], t2[:], iout[:], op=OP.mult)
    dene = pD.tile([P, L], F32, tag="dene")
    nc.vector.tensor_tensor(dene[:], t1[:], t2[:], op=OP.add)

    # ---------------- DMA out
    nc.sync.dma_start(dram["ui"][g], O[:].rearrange("p l r -> p (l r)"))
    nc.sync.dma_start(dram["uz"][g], Oz[:].rearrange("p l r -> p (l r)"))
    nc.sync.dma_start(dram["dnum"][g], dnum[:])
    nc.sync.dma_start(dram["lfin"][g], lfin[:])
    nc.sync.dma_start(dram["dene"][g], dene[:])
    nc.sync.dma_start(dram["ui0"][g], O[:, :, 0])


def build_nc():
    nc = bacc.Bacc("TRN2", target_bir_lowering=False, debug=False)
    ins = {
        "blob": nc.dram_tensor("blob", [P, 3104], F32,
                               kind="ExternalInput").ap(),
    }
    outs = {
        "ui": nc.dram_tensor("ui", [G, P, L * 900], F32,
                             kind="ExternalOutput").ap(),
        "uz": nc.dram_tensor("uz", [G, P, L * 100], F32,
                             kind="ExternalOutput").ap(),
        "dnum": nc.dram_tensor("dnum", [G, P, L], F32,
                               kind="ExternalOutput").ap(),
        "lfin": nc.dram_tensor("lfin", [G, P, L], F32,
                               kind="ExternalOutput").ap(),
        "dene": nc.dram_tensor("dene", [G, P, L], F32,
                               kind="ExternalOutput").ap(),
        "ui0": nc.dram_tensor("ui0", [G, P, L], F32,
                              kind="ExternalOutput").ap(),
    }
    with tile.TileContext(nc) as tc:
        with (tc.tile_pool(name="pconst", bufs=1) as pK,
              tc.tile_pool(name="pA", bufs=1) as pA,
              tc.tile_pool(name="pB", bufs=1) as pB,
              tc.tile_pool(name="pC", bufs=1) as pC,
              tc.tile_pool(name="pD", bufs=1) as pD):
            blob_t = pK.tile([P, 3104], F32, tag="blob")
            nc.sync.dma_start(blob_t[:], ins["blob"])
            for g in range(G):
                build_group(nc, (pA, pB, pC, pD), g, outs, blob_t[:])
    return nc


@functools.lru_cache(maxsize=1)
def _built():
    nc = build_nc()
    nc.compile()
    return nc


def host_inputs(init_energy):
    """Per-core input dicts. init_energy: (4096,) float32."""
    lv = np.arange(L, dtype=np.float64)
    r = np.linspace(RD, RM, RN, dtype=np.float32).astype(np.float64)
    arep = (lv[:, None] * (lv[:, None] + 1) / r[None] ** 2
            - 1.0 / r[None]).astype(np.float32)
    arep_rep = np.ascontiguousarray(
        np.broadcast_to(arep[None], (P, L, RN))).astype(np.float32)

    r0 = np.linspace(RD, 5 * RD, 5)
    uzi = (r0[None] ** (lv[:, None] + 1)
           - r0[None] ** (lv[:, None] + 2) / (2 * (lv[:, None] + 1))
           ).astype(np.float32)
    uzi_rep = np.ascontiguousarray(
        np.broadcast_to(uzi[None], (P, L, 5))).astype(np.float32)

    r_inf = np.linspace(RM - 4 * RD, RM, 5)
    fact = np.array([1.0, 3.0, 15.0])
    maps = []
    for core in range(NCORES):
        e_c = np.asarray(init_energy[core * NE_CORE:(core + 1) * NE_CORE],
                         dtype=np.float32)
        e_pg = np.ascontiguousarray(e_c.reshape(G, P).T)       # [P, G]
        x = (r_inf[None, None] *
             np.sqrt(np.abs(e_c.astype(np.float64)))[:, None, None])
        base = x ** lv[None, :, None] / fact[None, :, None]
        t1 = x ** 2 / 2.0 / (2 * lv[None, :, None] + 3)
        t2 = ((x ** 2 / 2.0) ** 2
              / (2.0 * (2 * lv[None, :, None] + 3) * (2 * lv[None, :, None] + 5)))
        uii = (r_inf[None, None] * (base * (1.0 + t1 + t2))).astype(np.float32)
        uii_pg = np.ascontiguousarray(
            uii.reshape(G, P, L, 5).transpose(1, 0, 2, 3))     # [P, G, L, 5]
        blob = np.zeros((P, 3104), np.float32)
        blob[:, 0:3000] = arep_rep.reshape(P, 3000)
        blob[:, 3000:3004] = e_pg
        blob[:, 3004:3064] = uii_pg.reshape(P, 60)
        blob[:, 3064:3079] = uzi_rep.reshape(P, 15)
        maps.append({"blob": blob})
    return maps


def assemble(init_energy, results):
    """results: list of per-core dicts of output arrays -> full outputs."""
    ui = np.stack([res["ui"].reshape(G, P, L, 900) for res in results])
    uz = np.stack([res["uz"].reshape(G, P, L, 100) for res in results])
    # [core, g, p, l, k] -> [k, core*g*p, l]
    u_infty = np.ascontiguousarray(
        ui.transpose(4, 0, 1, 2, 3).reshape(900, NE_FULL, L))
    u_zero = np.ascontiguousarray(
        uz.transpose(4, 0, 1, 2, 3).reshape(100, NE_FULL, L))

    def smalls(name):
        return np.concatenate(
            [res[name].reshape(G * P, L) for res in results], axis=0)

    dnum, lfin, dene, ui0 = (smalls(n) for n in
                             ("dnum", "lfin", "dene", "ui0"))
    lfout = (dnum[::-1, :] / ui0).astype(np.float32)
    delta = (-(lfout - lfin) / dene).astype(np.float32)
    e_out = (np.asarray(init_energy, np.float32)[:, None] + delta
             ).astype(np.float32)
    return e_out, u_zero, u_infty


def kernel(init_energy):
    nc = _built()
    in_maps = host_inputs(init_energy)
    res = bass_utils.run_bass_kernel_spmd(nc, in_maps,
                                          core_ids=list(range(NCORES)))
    return assemble(init_energy, res.results)
